# revision 1
# baseline (speedup 1.0000x reference)
"""HA_NET Trainium2 Bass kernel.

Hierarchical GRU net: word-level bi-GRU over 256 sentences x 256 words
(data-parallel, 32 sentences/core), conv head (widths 1-6, global max),
sentence-level bi-GRU (replicated scan), 3-layer MLP -> [1,1].

Layout strategy (per core):
- Gates-on-partitions: all GRU gate tensors live as [128(gate chunk), batch]
  so elementwise ops use full lanes and h'.T feeds the next matmul directly.
- gi (input-gate projections) precomputed with big matmuls, spilled to DRAM
  bf16, streamed back per scan step (DMA idle during the scan).
- All word-GRU states kept in SBUF (hall, bf16) for the conv phase.
- Conv = shifted accumulating matmuls into PSUM (has_written windowing),
  DVE max-reduce straight out of PSUM.
- All matmuls bf16 (validated: end-to-end rel err ~1e-6 vs fp32 reference).
"""

import os
import numpy as np

NCORES = 8
NS_TOT = 256      # total sentences
T_FULL = 256      # words per sentence
E = 300           # embedding
HWD = 256         # word GRU hidden
HS = 256          # sentence GRU hidden
G = 768           # 3 * hidden (gates r,z,n)
NC_F = 100        # conv filters per width
KWS = [1, 2, 3, 4, 5, 6]
PADS = {1: 0, 2: 0, 3: 1, 4: 1, 5: 2, 6: 2}


def build_program(S, T, n_cores):
    """Build the SPMD Bass program. S = sentences per core, T = words."""
    import concourse.bass as bass
    import concourse.bacc as bacc
    import concourse.tile as tile
    import concourse.mybir as mybir
    from concourse import masks
    from contextlib import ExitStack

    dt = mybir.dt
    f32, bf16 = dt.float32, dt.bfloat16
    AF = mybir.ActivationFunctionType
    Alu = mybir.AluOpType
    AX = mybir.AxisListType

    NS = S * n_cores          # total sentences
    P2 = 2 * T                # conv concat length
    NT = S * T                # word-positions per core
    # embedding K-chunks
    ECH = [128, 128, E - 256]
    GM = 6                    # gate chunks of 128

    nc = bacc.Bacc("TRN2", target_bir_lowering=False, debug=False,
                   num_devices=n_cores)

    # ---------------- DRAM I/O ----------------
    x_d = nc.dram_tensor("x_shard", [S, T, E], f32, kind="ExternalInput").ap()
    wih_w = nc.dram_tensor("wih_w", [G, E], f32, kind="ExternalInput").ap()
    whh_w = nc.dram_tensor("whh_w", [G, HWD], f32, kind="ExternalInput").ap()
    bih_w = nc.dram_tensor("bih_w", [G], f32, kind="ExternalInput").ap()
    bhh_w = nc.dram_tensor("bhh_w", [G], f32, kind="ExternalInput").ap()
    wih_s = nc.dram_tensor("wih_s", [G, 6 * NC_F], f32, kind="ExternalInput").ap()
    whh_s = nc.dram_tensor("whh_s", [G, HS], f32, kind="ExternalInput").ap()
    bih_s = nc.dram_tensor("bih_s", [G], f32, kind="ExternalInput").ap()
    bhh_s = nc.dram_tensor("bhh_s", [G], f32, kind="ExternalInput").ap()
    conv_w = {k: nc.dram_tensor(f"conv{k}_w", [NC_F, 1, k, HWD], f32,
                                kind="ExternalInput").ap() for k in KWS}
    conv_b = {k: nc.dram_tensor(f"conv{k}_b", [NC_F], f32,
                                kind="ExternalInput").ap() for k in KWS}
    fc1_w = nc.dram_tensor("fc1_w", [128, HS], f32, kind="ExternalInput").ap()
    fc1_b = nc.dram_tensor("fc1_b", [128], f32, kind="ExternalInput").ap()
    fc2_w = nc.dram_tensor("fc2_w", [32, 128], f32, kind="ExternalInput").ap()
    fc2_b = nc.dram_tensor("fc2_b", [32], f32, kind="ExternalInput").ap()
    fc3_w = nc.dram_tensor("fc3_w", [1, 32], f32, kind="ExternalInput").ap()
    fc3_b = nc.dram_tensor("fc3_b", [1], f32, kind="ExternalInput").ap()
    out_d = nc.dram_tensor("out", [1, 1], f32, kind="ExternalOutput").ap()

    # internal DRAM
    gi_d = nc.dram_tensor("gi_spill", [GM, 128, T, S], bf16, kind="Internal").ap()
    feats_loc = nc.dram_tensor("feats_local", [NC_F, 6, S], bf16,
                               kind="Internal").ap()
    feats_gat = nc.dram_tensor("feats_gathered", [n_cores, NC_F, 6, S], bf16,
                               kind="Internal", addr_space="Shared").ap()

    with tile.TileContext(nc) as tc, ExitStack() as ctx:
        # ---------------- persistent pools ----------------
        const = ctx.enter_context(tc.tile_pool(name="const", bufs=1))

        ident = const.tile([128, 128], f32)
        masks.make_identity(nc, ident[:])

        # stationary weights (bf16, pre-transposed)
        whhT = const.tile([128, 12 * 128], bf16)        # [kc*6+m]
        wihT = const.tile([128, 18 * 128], bf16)        # [kc*6+m] (kc rows: 128/128/44)
        wihsT = const.tile([128, 36 * 128], bf16)       # [(m*6+k)] rows :100
        whhsT = const.tile([128, 12 * 128], bf16)
        convwT = const.tile([128, 42 * NC_F], bf16)     # [(k,dk,kc)] packed
        fc1T = const.tile([128, 2 * 128], bf16)
        fc2T = const.tile([128, 32], bf16)
        fc3T = const.tile([32, 1], bf16)

        biases_w = const.tile([128, 6], f32)   # word: m<4: bih+bhh ; m>=4: bih
        bhh_w_sb = const.tile([128, 6], f32)
        bih_w_sb = const.tile([128, 6], f32)
        biases_s = const.tile([128, 6], f32)   # sentence, same structure
        bhh_s_sb = const.tile([128, 6], f32)
        bih_s_sb = const.tile([128, 6], f32)
        convb_sb = const.tile([NC_F, 6], f32)
        fc1b_sb = const.tile([128, 1], f32)
        fc2b_sb = const.tile([32, 1], f32)
        fc3b_sb = const.tile([1, 1], f32)

        # ---------------- P0: weight prep ----------------
        # transpose helper: dram [r, c] fp32 -> dest bf16 [c, r] slice
        p0_ctx = ExitStack()
        p0 = p0_ctx.enter_context(tc.tile_pool(name="p0stage", bufs=4))
        p0ps = p0_ctx.enter_context(tc.tile_pool(name="p0psum", bufs=4,
                                                 space="PSUM"))
        _alt = [0]

        def transp(dst_ap, src_ap, rr, cc):
            # src_ap: [rr, cc] fp32 in DRAM ; dst_ap: [cc, rr] bf16 SBUF slice
            st = p0.tile([128, 320], f32, tag="p0st")
            nc.sync.dma_start(out=st[:rr, :cc], in_=src_ap)
            ps = p0ps.tile([128, 128], f32, tag="p0ps")
            nc.tensor.matmul(ps[:cc, :rr], st[:rr, :cc], ident[:rr, :rr],
                             is_transpose=True)
            # alternate copy engine
            if _alt[0] % 2 == 0:
                nc.scalar.copy(dst_ap, ps[:cc, :rr])
            else:
                nc.vector.tensor_copy(dst_ap, ps[:cc, :rr])
            _alt[0] += 1

        whhT_v = whhT[:].rearrange("p (i q) -> p i q", q=128)
        for kc in range(2):
            for m in range(GM):
                transp(whhT_v[:, kc * 6 + m, :],
                       whh_w[m * 128:(m + 1) * 128, kc * 128:(kc + 1) * 128],
                       128, 128)
        wihT_v = wihT[:].rearrange("p (i q) -> p i q", q=128)
        for kc in range(3):
            cs = ECH[kc]
            for m in range(GM):
                transp(wihT_v[:cs, kc * 6 + m, :],
                       wih_w[m * 128:(m + 1) * 128, kc * 128:kc * 128 + cs],
                       128, cs)
        wihsT_v = wihsT[:].rearrange("p (i q) -> p i q", q=128)
        for m in range(GM):
            for k in range(6):
                transp(wihsT_v[:NC_F, m * 6 + k, :],
                       wih_s[m * 128:(m + 1) * 128, k * NC_F:(k + 1) * NC_F],
                       128, NC_F)
        whhsT_v = whhsT[:].rearrange("p (i q) -> p i q", q=128)
        for kc in range(2):
            for m in range(GM):
                transp(whhsT_v[:, kc * 6 + m, :],
                       whh_s[m * 128:(m + 1) * 128, kc * 128:(kc + 1) * 128],
                       128, 128)
        convwT_v = convwT[:].rearrange("p (i q) -> p i q", q=NC_F)
        conv_idx = {}
        ci = 0
        for k in KWS:
            for dk in range(k):
                for kc in range(2):
                    conv_idx[(k, dk, kc)] = ci
                    transp(convwT_v[:, ci, :],
                           conv_w[k][:, 0, dk, kc * 128:(kc + 1) * 128],
                           NC_F, 128)
                    ci += 1
        fc1T_v = fc1T[:].rearrange("p (i q) -> p i q", q=128)
        for kc in range(2):
            transp(fc1T_v[:, kc, :], fc1_w[:, kc * 128:(kc + 1) * 128], 128, 128)
        transp(fc2T[:, :], fc2_w[:, :], 32, 128)
        transp(fc3T[:, :], fc3_w[:, :], 1, 32)

        # biases
        nc.sync.dma_start(out=bih_w_sb[:], in_=bih_w.rearrange("(m p) -> p m", p=128))
        nc.sync.dma_start(out=bhh_w_sb[:], in_=bhh_w.rearrange("(m p) -> p m", p=128))
        nc.sync.dma_start(out=bih_s_sb[:], in_=bih_s.rearrange("(m p) -> p m", p=128))
        nc.sync.dma_start(out=bhh_s_sb[:], in_=bhh_s.rearrange("(m p) -> p m", p=128))
        nc.vector.tensor_add(biases_w[:, 0:4], bih_w_sb[:, 0:4], bhh_w_sb[:, 0:4])
        nc.vector.tensor_copy(biases_w[:, 4:6], bih_w_sb[:, 4:6])
        nc.vector.tensor_add(biases_s[:, 0:4], bih_s_sb[:, 0:4], bhh_s_sb[:, 0:4])
        nc.vector.tensor_copy(biases_s[:, 4:6], bih_s_sb[:, 4:6])
        for j, k in enumerate(KWS):
            nc.sync.dma_start(out=convb_sb[:, j:j + 1],
                              in_=conv_b[k][:, None])
        nc.sync.dma_start(out=fc1b_sb[:], in_=fc1_b[:, None])
        nc.sync.dma_start(out=fc2b_sb[:], in_=fc2_b[:, None])
        nc.sync.dma_start(out=fc3b_sb[:], in_=fc3_b[:, None])
        p0_ctx.close()

        # ---------------- P1+P2: x transpose and gi = x @ wih.T ----------------
        # xT: [c-chunk][128, NT] bf16, t-major columns (t*S + s)
        with tc.tile_pool(name="xT", bufs=1) as xtp, \
             tc.tile_pool(name="p1stage", bufs=4) as p1s, \
             tc.tile_pool(name="p1psum", bufs=4, space="PSUM") as p1p, \
             tc.tile_pool(name="gipsum", bufs=4, space="PSUM") as gip, \
             tc.tile_pool(name="gistage", bufs=4) as gis:
            xT = [xtp.tile([128, NT], bf16, tag=f"xT{j}", name=f"xT{j}")
                  for j in range(3)]
            tpb = 128 // S  # t's per 128-row block
            nblk = NT // 128
            for i in range(nblk):
                st = p1s.tile([128, E], f32, tag="xst")
                nc.sync.dma_start(
                    out=st[:],
                    in_=x_d[:, i * tpb:(i + 1) * tpb, :].rearrange(
                        "s t c -> t s c"))
                for j in range(3):
                    cs = ECH[j]
                    ps = p1p.tile([128, 128], f32, tag="xps")
                    nc.tensor.matmul(ps[:cs, :], st[:, j * 128:j * 128 + cs],
                                     ident[:], is_transpose=True)
                    if i % 2 == 0:
                        nc.scalar.copy(xT[j][:cs, i * 128:(i + 1) * 128],
                                       ps[:cs, :])
                    else:
                        nc.vector.tensor_copy(
                            xT[j][:cs, i * 128:(i + 1) * 128], ps[:cs, :])

            # gi matmuls -> DRAM spill (bias folded in copy)
            NJ = max(1, NT // 512)
            njw = NT // NJ
            tpj = njw // S
            for m in range(GM):
                for nj in range(NJ):
                    ps = gip.tile([128, njw], f32, tag="gips")
                    for kc in range(3):
                        cs = ECH[kc]
                        nc.tensor.matmul(
                            ps[:], wihT_v[:cs, kc * 6 + m, :],
                            xT[kc][:cs, nj * njw:(nj + 1) * njw],
                            start=(kc == 0), stop=(kc == 2))
                    stg = gis.tile([128, njw], bf16, tag="gistg")
                    if (m * NJ + nj) % 2 == 0:
                        nc.scalar.activation(stg[:], ps[:], AF.Identity,
                                             bias=biases_w[:, m:m + 1])
                    else:
                        nc.vector.tensor_scalar_add(stg[:], ps[:],
                                                    biases_w[:, m:m + 1])
                    nc.sync.dma_start(
                        out=gi_d[m, :, nj * tpj:(nj + 1) * tpj, :], in_=stg[:])

        # ---------------- P3: word-level bi-GRU scan ----------------
        # hall: [128, c(2), s(S), pos(2T+2)] bf16
        # fwd state t -> pos t+1 (pos 0 zero) ; bwd state t -> pos t+257
        # (pos 2T+1 zero). conv reads pos 1..2T+1 contiguously.
        QP = 2 * T + 2
        with tc.tile_pool(name="hall", bufs=1) as hallp:
            hall = hallp.tile([128, 2 * S * QP], bf16)
            hv = hall[:].rearrange("p (c s q) -> p c s q", c=2, s=S, q=QP)
            nc.gpsimd.memset(hv[:, :, :, 0:1], 0.0)
            nc.gpsimd.memset(hv[:, :, :, QP - 1:QP], 0.0)

            with tc.tile_pool(name="scanps", bufs=2, space="PSUM") as scanps, \
                 tc.tile_pool(name="scansb", bufs=3) as scansb, \
                 tc.tile_pool(name="gistep", bufs=6) as gistep:

                def word_step(t, pos_prev, pos_new, tag):
                    gi_t = gistep.tile([128, 6 * S], bf16, tag=f"gi{tag}")
                    gv = gi_t[:].rearrange("p (m s) -> p m s", m=6)
                    nc.sync.dma_start(
                        out=gv[:, :, :],
                        in_=gi_d[:, :, t, :].rearrange("m p s -> p m s"))
                    ps_rz = scanps.tile([128, 4 * S], f32, tag=f"rz{tag}")
                    ps_n = scanps.tile([128, 2 * S], f32, tag=f"n{tag}")
                    rzv = ps_rz[:].rearrange("p (m s) -> p m s", m=4)
                    nv = ps_n[:].rearrange("p (m s) -> p m s", m=2)
                    for m in range(GM):
                        dst = rzv[:, m, :] if m < 4 else nv[:, m - 4, :]
                        for kc in range(2):
                            nc.tensor.matmul(dst, whhT_v[:, kc * 6 + m, :],
                                             hv[:, kc, :, pos_prev],
                                             start=(kc == 0), stop=(kc == 1))
                    # rz pre-act += gi ; sigmoid
                    nc.vector.tensor_add(rzv[:, :, :], rzv[:, :, :],
                                         gv[:, 0:4, :])
                    rz_sb = scansb.tile([128, 4 * S], bf16, tag=f"rzs{tag}")
                    rzs = rz_sb[:].rearrange("p (m s) -> p m s", m=4)
                    nc.scalar.activation(rzs[:, :, :], rzv[:, :, :], AF.Sigmoid)
                    # n pre-act: (gh_n + bhh_n) * r  + gi_n ; tanh
                    nmix = scansb.tile([128, 2 * S], f32, tag=f"nm{tag}")
                    nmv = nmix[:].rearrange("p (m s) -> p m s", m=2)
                    for mi in range(2):
                        nc.vector.scalar_tensor_tensor(
                            nmv[:, mi, :], nv[:, mi, :],
                            bhh_w_sb[:, 4 + mi:5 + mi], rzs[:, mi, :],
                            op0=Alu.add, op1=Alu.mult)
                    nc.vector.tensor_add(nv[:, :, :], nmv[:, :, :],
                                         gv[:, 4:6, :])
                    n_sb = scansb.tile([128, 2 * S], bf16, tag=f"ns{tag}")
                    nsv = n_sb[:].rearrange("p (m s) -> p m s", m=2)
                    nc.scalar.activation(nsv[:, :, :], nv[:, :, :], AF.Tanh)
                    # h' = n + z*(h - n)
                    d_sb = scansb.tile([128, 2 * S], bf16, tag=f"ds{tag}")
                    dv = d_sb[:].rearrange("p (m s) -> p m s", m=2)
                    nc.gpsimd.tensor_sub(dv[:, :, :], hv[:, :, :, pos_prev],
                                         nsv[:, :, :])
                    zd_sb = scansb.tile([128, 2 * S], bf16, tag=f"zd{tag}")
                    zdv = zd_sb[:].rearrange("p (m s) -> p m s", m=2)
                    nc.gpsimd.tensor_mul(zdv[:, :, :], rzs[:, 2:4, :],
                                         dv[:, :, :])
                    nc.gpsimd.tensor_add(hv[:, :, :, pos_new], zdv[:, :, :],
                                         nsv[:, :, :])

                for i in range(T):
                    word_step(i, i, i + 1, "f")
                    tb = T - 1 - i
                    word_step(tb, tb + T + 2, tb + T + 1, "b")

            # ---------------- P4: conv head + max + sigmoid ----------------
            maxsb = const.tile([NC_F, 6 * S], f32)
            mxv = maxsb[:].rearrange("p (k s) -> p k s", k=6)
            featsT = const.tile([NC_F, 6 * S], bf16)
            ftv = featsT[:].rearrange("p (k s) -> p k s", k=6)
            with tc.tile_pool(name="convps", bufs=8, space="PSUM") as convps:
                for ki, k in enumerate(KWS):
                    pad = PADS[k]
                    T_out = P2 - k + 1 + 2 * pad
                    dks = [pad] + [d for d in range(k) if d != pad]
                    for sg in range(0, S, 8):
                        gsz = min(8, S - sg)
                        pcs = [convps.tile([128, 512], f32, tag="cps",
                                            name=f"cps{si}")
                               for si in range(gsz)]
                        for di, dk in enumerate(dks):
                            dlt = dk - pad
                            t0 = max(0, -dlt)
                            t1 = min(T_out, P2 - dlt)
                            for kc in range(2):
                                w_ap = convwT_v[:, conv_idx[(k, dk, kc)], :]
                                st = (di == 0 and kc == 0)
                                sp = (di == len(dks) - 1 and kc == 1)
                                for si in range(gsz):
                                    s = sg + si
                                    nc.tensor.matmul(
                                        pcs[si][:NC_F, t0:t1], w_ap,
                                        hv[:, kc, s, 1 + t0 + dlt:1 + t1 + dlt],
                                        start=st, stop=sp)
                        for si in range(gsz):
                            nc.vector.tensor_reduce(
                                mxv[:NC_F, ki, sg + si:sg + si + 1],
                                pcs[si][:NC_F, 0:T_out], axis=AX.X, op=Alu.max)
                    nc.scalar.activation(ftv[:NC_F, ki, :], mxv[:NC_F, ki, :],
                                         AF.Sigmoid,
                                         bias=convb_sb[:NC_F, ki:ki + 1])

            # ---------------- P5: AllGather feats ----------------
            nc.sync.dma_start(out=feats_loc[:, :, :],
                              in_=ftv[:NC_F, :, :])
            nc.gpsimd.collective_compute(
                "AllGather", Alu.bypass,
                replica_groups=[list(range(n_cores))],
                ins=[feats_loc[:, :, :]],
                outs=[feats_gat[:, :, :, :]])
            featsk = const.tile([NC_F, 6 * NS], bf16)
            fkv = featsk[:].rearrange("p (k n) -> p k n", k=6)
            nc.sync.dma_start(
                out=fkv[:NC_F, :, :],
                in_=feats_gat.rearrange("co o k s -> o k co s"))

            # ---------------- P6: gi_s = feats @ wih_s.T (+biases) ----------------
            gi_sT = const.tile([128, 6 * NS], f32)
            gsv = gi_sT[:].rearrange("p (m n) -> p m n", m=6)
            with tc.tile_pool(name="gisps", bufs=3, space="PSUM") as gisps:
                for m in range(GM):
                    ps = gisps.tile([128, NS], f32, tag="gisps")
                    for k in range(6):
                        nc.tensor.matmul(ps[:], wihsT_v[:NC_F, m * 6 + k, :],
                                         fkv[:NC_F, k, :],
                                         start=(k == 0), stop=(k == 5))
                    nc.scalar.activation(gsv[:, m, :], ps[:], AF.Identity,
                                         bias=biases_s[:, m:m + 1])

            # ---------------- P7: sentence bi-GRU (replicated) ----------------
            # hs_all: [128, c(2), d(2), NS+1] bf16
            # fwd (d=0): state t -> pos t+1, init pos 0
            # bwd (d=1): state t -> pos t, init pos NS
            hs_all = const.tile([128, 2 * 2 * (NS + 1)], bf16)
            hsv = hs_all[:].rearrange("p (c d q) -> p c d q", c=2, d=2)
            nc.gpsimd.memset(hsv[:, :, 0, 0:1], 0.0)
            nc.gpsimd.memset(hsv[:, :, 1, NS:NS + 1], 0.0)
            with tc.tile_pool(name="sps", bufs=2, space="PSUM") as sps, \
                 tc.tile_pool(name="ssb", bufs=3) as ssb:

                def sent_step(t, d, pos_prev, pos_new, tag):
                    ps = sps.tile([128, 6], f32, tag=f"sp{tag}")
                    for m in range(GM):
                        for kc in range(2):
                            nc.tensor.matmul(
                                ps[:, m:m + 1], whhsT_v[:, kc * 6 + m, :],
                                hsv[:, kc, d, pos_prev:pos_prev + 1],
                                start=(kc == 0), stop=(kc == 1))
                    rz = ssb.tile([128, 4], bf16, tag=f"srz{tag}")
                    for mi in range(4):
                        nc.scalar.activation(rz[:, mi:mi + 1], ps[:, mi:mi + 1],
                                             AF.Sigmoid,
                                             bias=gsv[:, mi, t:t + 1])
                    nm = ssb.tile([128, 2], f32, tag=f"snm{tag}")
                    for mi in range(2):
                        nc.vector.scalar_tensor_tensor(
                            nm[:, mi:mi + 1], ps[:, 4 + mi:5 + mi],
                            bhh_s_sb[:, 4 + mi:5 + mi], rz[:, mi:mi + 1],
                            op0=Alu.add, op1=Alu.mult)
                    n_t = ssb.tile([128, 2], bf16, tag=f"sn{tag}")
                    for mi in range(2):
                        nc.scalar.activation(n_t[:, mi:mi + 1],
                                             nm[:, mi:mi + 1], AF.Tanh,
                                             bias=gsv[:, 4 + mi, t:t + 1])
                    dd = ssb.tile([128, 2], bf16, tag=f"sd{tag}")
                    nc.gpsimd.tensor_sub(dd[:, :],
                                         hsv[:, :, d, pos_prev],
                                         n_t[:, :])
                    zd = ssb.tile([128, 2], bf16, tag=f"szd{tag}")
                    nc.gpsimd.tensor_mul(zd[:, :], rz[:, 2:4], dd[:, :])
                    nc.gpsimd.tensor_add(hsv[:, :, d, pos_new], zd[:, :],
                                         n_t[:, :])

                for i in range(NS):
                    sent_step(i, 0, i, i + 1, "f")
                    tb = NS - 1 - i
                    sent_step(tb, 1, tb + 1, tb, "b")

            # ---------------- P8: means + MLP ----------------
            with tc.tile_pool(name="mlpps", bufs=2, space="PSUM") as mlpps, \
                 tc.tile_pool(name="mlpsb", bufs=2) as mlpsb:
                sums = mlpsb.tile([128, 4], f32, tag="sums")
                nc.vector.tensor_reduce(sums[:, 0:2], hsv[:, :, 0, 1:NS + 1],
                                        axis=AX.X, op=Alu.add)
                nc.vector.tensor_reduce(sums[:, 2:4], hsv[:, :, 1, 0:NS],
                                        axis=AX.X, op=Alu.add)
                hdoc = mlpsb.tile([128, 2], bf16, tag="hdoc")
                hdf = mlpsb.tile([128, 2], f32, tag="hdf")
                nc.vector.tensor_add(hdf[:, :], sums[:, 0:2], sums[:, 2:4])
                nc.vector.tensor_scalar_mul(hdoc[:, :], hdf[:, :],
                                            0.5 / NS)
                ps1 = mlpps.tile([128, 1], f32, tag="ps1")
                for kc in range(2):
                    nc.tensor.matmul(ps1[:, :], fc1T_v[:, kc, :],
                                     hdoc[:, kc:kc + 1],
                                     start=(kc == 0), stop=(kc == 1))
                x1 = mlpsb.tile([128, 1], bf16, tag="x1")
                nc.scalar.activation(x1[:, :], ps1[:, :], AF.Sigmoid,
                                     bias=fc1b_sb[:, :])
                ps2 = mlpps.tile([128, 1], f32, tag="ps2")
                nc.tensor.matmul(ps2[:32, :], fc2T[:, :], x1[:, :])
                x2 = mlpsb.tile([32, 1], bf16, tag="x2")
                nc.scalar.activation(x2[:, :], ps2[:32, :], AF.Sigmoid,
                                     bias=fc2b_sb[:, :])
                ps3 = mlpps.tile([128, 1], f32, tag="ps3")
                nc.tensor.matmul(ps3[:1, :], fc3T[:, :], x2[:, :])
                res = mlpsb.tile([1, 1], f32, tag="res")
                nc.scalar.activation(res[:, :], ps3[:1, :], AF.Sigmoid,
                                     bias=fc3b_sb[:, :])
                nc.sync.dma_start(out=out_d[:, :], in_=res[:, :])

    nc.compile()
    return nc


_PROGRAM_CACHE = {}


def _get_program(S, T, n_cores):
    key = (S, T, n_cores)
    if key not in _PROGRAM_CACHE:
        _PROGRAM_CACHE[key] = build_program(S, T, n_cores)
    return _PROGRAM_CACHE[key]


def kernel(**inputs):
    from concourse.bass_utils import run_bass_kernel_spmd

    x = np.ascontiguousarray(np.asarray(inputs["inputs_all"], dtype=np.float32))
    ns, T, _ = x.shape
    S = ns // NCORES
    nc = _get_program(S, T, NCORES)

    weights = {k: np.ascontiguousarray(np.asarray(v, dtype=np.float32))
               for k, v in inputs.items() if k != "inputs_all"}
    in_maps = []
    for c in range(NCORES):
        m = {"x_shard": np.ascontiguousarray(x[c * S:(c + 1) * S])}
        m.update(weights)
        in_maps.append(m)
    res = run_bass_kernel_spmd(nc, in_maps, list(range(NCORES)))
    return np.asarray(res.results[0]["out"], dtype=np.float32)

